# revision 1
# baseline (speedup 1.0000x reference)
"""
Trainium2 Bass kernel for batched cross-attention:
  context[b] = softmax(q[b] @ tokens[b].T / sqrt(d)) @ tokens[b]
with x_latent (tokens) [16, 4096, 768] f32, prompts_latent (q) [16, 64, 768] f32.

Sharding: data-parallel over the batch dim — 16 batches / 8 cores = 2 per core.

Per-core algorithm (bf16 matmuls, f32 accumulation):
  - host pre-transposes: qT [768, 64] and T^T [768, 4096] (both bf16), and
    ships tokens in natural layout T [4096, 768] bf16 as well.
  - mm1: S[64, 512-group] = qT.T @ T^T-slice, accumulated over 6 d-chunks.
  - softmax without max-subtraction (scores ~ N(0,1) after scaling; exp is
    safe): P = exp(S * scale) on ACT in [64, 128] chunks; row sums on DVE.
  - P chunks [64, 128] are PE-transposed to P^T [128, 64] (the second matmul
    contracts over n, which must be the partition dim).
  - mm2: O[64, 768] += P^T-tile.T @ T-tile, accumulated over 32 n-tiles.
  - O rows divided by the softmax sums at the end (DVE), stored as f32.

The group loop is software-pipelined two stages deep (PE program order per
iteration: mm1(g), transposes(g-1), mm2(g-2)) so the PE never waits on the
ACT exp or the DVE copies.
"""

import os
import sys

import numpy as np

for _p in ("/opt/trn_rl_repo", "/root/.axon_site/_ro/trn_rl_repo"):
    if os.path.isdir(_p) and _p not in sys.path:
        sys.path.append(_p)

import ml_dtypes
from contextlib import ExitStack

import concourse.bass as bass
import concourse.mybir as mybir
import concourse.tile as tile
from concourse import bacc
from concourse.bass_utils import run_bass_kernel_spmd
from concourse.masks import make_identity

BF16 = ml_dtypes.bfloat16

N_CORES = 8
B_TOTAL = 16
BPC = B_TOTAL // N_CORES  # batches per core
N = 4096  # tokens
D = 768   # latent dim
P = 64    # prompts
DC = D // 128   # d-chunks of 128 (contraction tiles for mm1)
NT = N // 128   # n-tiles of 128
G = N // 512    # groups of 512 columns for mm1/softmax
SCALE = float(D) ** -0.5

_cached_nc = None


def build_bass_program() -> bass.Bass:
    nc = bacc.Bacc("TRN2", target_bir_lowering=False, debug=False)
    qt = nc.declare_dram_parameter("qt", [BPC, D, P], mybir.dt.bfloat16, isOutput=False)
    tt = nc.declare_dram_parameter("tt", [BPC, D, N], mybir.dt.bfloat16, isOutput=False)
    tn = nc.declare_dram_parameter("tn", [BPC, N, D], mybir.dt.bfloat16, isOutput=False)
    out = nc.declare_dram_parameter("out", [BPC, P, D], mybir.dt.float32, isOutput=True)

    with tile.TileContext(nc) as tc, ExitStack() as ctx:
        singles = ctx.enter_context(tc.tile_pool(name="singles", bufs=1))
        qt_pool = ctx.enter_context(tc.tile_pool(name="qt", bufs=3))
        tt_pool = ctx.enter_context(tc.tile_pool(name="ttg", bufs=6))
        tn_pool = ctx.enter_context(tc.tile_pool(name="tnt", bufs=7))
        p_pool = ctx.enter_context(tc.tile_pool(name="pexp", bufs=4))
        pt_pool = ctx.enter_context(tc.tile_pool(name="ptT", bufs=12))
        sums_pool = ctx.enter_context(tc.tile_pool(name="sums", bufs=2))
        o_pool = ctx.enter_context(tc.tile_pool(name="osb", bufs=2))

        psum_s = ctx.enter_context(tc.tile_pool(name="psum_s", bufs=3, space="PSUM"))
        psum_pt = ctx.enter_context(tc.tile_pool(name="psum_pt", bufs=3, space="PSUM"))
        psum_o = ctx.enter_context(tc.tile_pool(name="psum_o", bufs=1, space="PSUM"))

        ident = singles.tile([P, P], mybir.dt.bfloat16)
        make_identity(nc, ident)

        # Per-batch state; o accumulators allocated lazily at first mm2 so
        # batch 1's PSUM allocation doesn't wait on batch 0's release.
        qt_ts = [None] * BPC
        sums_t = [None] * BPC
        o_ab = [None] * BPC

        def transpose_stage(p_sb, b, g):
            # PE transposes of the 4 P chunks + DVE copies to SBUF.
            pts = []
            for j in range(4):
                pt_ps = psum_pt.tile([128, P], mybir.dt.bfloat16)
                nc.tensor.transpose(pt_ps, p_sb[:, j * 128:(j + 1) * 128], ident)
                pts.append(pt_ps)
            outs = []
            for j in range(4):
                pt_sb = pt_pool.tile([128, P], mybir.dt.bfloat16)
                nc.vector.tensor_copy(pt_sb, pts[j])
                outs.append(pt_sb)
            return outs

        def mm2_stage(pt_sbs, tn_g, b, g):
            if o_ab[b] is None:
                o_a = psum_o.tile([P, 512], mybir.dt.float32, tag="o_a")
                o_b_ = psum_o.tile([P, 256], mybir.dt.float32, tag="o_b")
                o_ab[b] = (o_a, o_b_)
            o_a, o_b_ = o_ab[b]
            for j in range(4):
                nt = g * 4 + j
                nc.tensor.matmul(
                    o_a,
                    lhsT=pt_sbs[j],
                    rhs=tn_g[:, j, 0:512],
                    start=(nt == 0),
                    stop=(nt == NT - 1),
                )
                nc.tensor.matmul(
                    o_b_,
                    lhsT=pt_sbs[j],
                    rhs=tn_g[:, j, 512:768],
                    start=(nt == 0),
                    stop=(nt == NT - 1),
                )
            if g == G - 1:
                finish_batch(b)

        def finish_batch(b):
            # normalization + store; emitted immediately after the batch's
            # last mm2 so its PSUM accumulators release quickly.
            tot = sums_pool.tile([P, 1], mybir.dt.float32)
            nc.vector.reduce_sum(tot, sums_t[b], axis=mybir.AxisListType.X)
            rec = sums_pool.tile([P, 1], mybir.dt.float32)
            nc.vector.reciprocal(rec, tot)
            o_a, o_b_ = o_ab[b]
            o_sb = o_pool.tile([P, D], mybir.dt.float32)
            nc.vector.tensor_scalar_mul(o_sb[:, 0:512], o_a, rec)
            nc.vector.tensor_scalar_mul(o_sb[:, 512:768], o_b_, rec)
            nc.sync.dma_start(out=out[b], in_=o_sb)

        # One continuous two-stage software pipeline across BOTH batches:
        # PE program order per iteration is [mm1(i)] [mm2(i-2)]
        # [transposes(i-1)] — no pipeline flush (and no PE/DMA stall) at the
        # batch boundary.
        tr_q = []   # (p_sb, tn_g, b, g) awaiting transpose stage (depth 2)
        mm2_q = []  # (pt_sbs, tn_g, b, g) awaiting mm2 stage
        for idx in range(BPC * G):
            b, g = divmod(idx, G)
            if g == 0:
                qt_ts[b] = qt_pool.tile([128, DC, P], mybir.dt.bfloat16, tag="qt_t", name="qt_t")
                nc.sync.dma_start(
                    out=qt_ts[b], in_=qt[b].rearrange("(c p) m -> p c m", p=128)
                )
                sums_t[b] = sums_pool.tile([P, G], mybir.dt.float32, tag="sums", name="sums")
            qt_t = qt_ts[b]
            tt_r = tt[b].rearrange("(c p) n -> p c n", p=128)
            tn_r = tn[b].rearrange("(g t p) d -> p g t d", t=4, p=128)

            tt_g = tt_pool.tile([128, DC, 512], mybir.dt.bfloat16)
            if idx == 0:
                # Split the very first tt load per d-chunk so the first mm1
                # matmul only waits for the first 128KB instead of the whole
                # first-load burst (observed ~4-5us dead time at kernel head).
                for c in range(DC):
                    nc.sync.dma_start(
                        out=tt_g[:, c, :], in_=tt_r[:, c, g * 512:(g + 1) * 512]
                    )
            else:
                nc.sync.dma_start(out=tt_g, in_=tt_r[:, :, g * 512:(g + 1) * 512])

            tn_g = tn_pool.tile([128, 4, D], mybir.dt.bfloat16)
            if idx >= 2:
                nc.sync.dma_start(out=tn_g, in_=tn_r[:, g])

            s_ps = psum_s.tile([P, 512], mybir.dt.float32)
            for c in range(DC):
                nc.tensor.matmul(
                    s_ps,
                    lhsT=qt_t[:, c, :],
                    rhs=tt_g[:, c, :],
                    start=(c == 0),
                    stop=(c == DC - 1),
                )

            # P = exp(S * scale), cast to bf16. Chunked so the PE transposes
            # can start after the first 128 columns instead of waiting out
            # the full-width activation. Row sums on DVE.
            p_sb = p_pool.tile([P, 512], mybir.dt.bfloat16)
            for j in range(4):
                nc.scalar.activation(
                    out=p_sb[:, j * 128:(j + 1) * 128],
                    in_=s_ps[:, j * 128:(j + 1) * 128],
                    func=mybir.ActivationFunctionType.Exp,
                    scale=SCALE,
                )
            nc.vector.reduce_sum(
                sums_t[b][:, g:g + 1], p_sb, axis=mybir.AxisListType.X
            )

            if idx < 2:
                # tn isn't needed until two iterations later; issuing the
                # first two after mm1/exp keeps the head matmuls' DMA-queue
                # waits from covering them.
                nc.sync.dma_start(out=tn_g, in_=tn_r[:, g])

            if len(tr_q) == 2:
                if mm2_q:
                    mm2_stage(*mm2_q.pop(0))
                p_sb0, tn_g0, b0, g0 = tr_q.pop(0)
                pt_sbs = transpose_stage(p_sb0, b0, g0)
                mm2_q.append((pt_sbs, tn_g0, b0, g0))
            tr_q.append((p_sb, tn_g, b, g))
        while tr_q:
            if mm2_q:
                mm2_stage(*mm2_q.pop(0))
            p_sb0, tn_g0, b0, g0 = tr_q.pop(0)
            pt_sbs = transpose_stage(p_sb0, b0, g0)
            mm2_q.append((pt_sbs, tn_g0, b0, g0))
        while mm2_q:
            mm2_stage(*mm2_q.pop(0))

    nc.compile()
    return nc


def _get_nc() -> bass.Bass:
    global _cached_nc
    if _cached_nc is None:
        _cached_nc = build_bass_program()
    return _cached_nc


def _make_in_maps(x_latent: np.ndarray, prompts_latent: np.ndarray):
    tn_h = np.ascontiguousarray(x_latent.astype(BF16))            # [16, N, D]
    tt_h = np.ascontiguousarray(tn_h.transpose(0, 2, 1))          # [16, D, N]
    qt_h = np.ascontiguousarray(prompts_latent.astype(BF16).transpose(0, 2, 1))
    return [
        {
            "qt": qt_h[c * BPC:(c + 1) * BPC],
            "tt": tt_h[c * BPC:(c + 1) * BPC],
            "tn": tn_h[c * BPC:(c + 1) * BPC],
        }
        for c in range(N_CORES)
    ]


def run(x_latent: np.ndarray, prompts_latent: np.ndarray, trace: bool = False):
    """Run on all 8 cores; returns (output [16, 64, 768] f32, BassKernelResults)."""
    nc = _get_nc()
    in_maps = _make_in_maps(np.asarray(x_latent), np.asarray(prompts_latent))
    res = run_bass_kernel_spmd(nc, in_maps, list(range(N_CORES)), trace=trace)
    out = np.concatenate([np.asarray(r["out"]) for r in res.results], axis=0)
    return out.astype(np.float32), res


def kernel(x_latent: np.ndarray, prompts_latent: np.ndarray) -> np.ndarray:
    out, _ = run(x_latent, prompts_latent, trace=False)
    return out

